# revision 1
# baseline (speedup 1.0000x reference)
"""GAT layer (gnn_message_passing) Trainium2 Bass kernel.

Per-core work (data-parallel over batch B=8, one graph per NeuronCore):
  h   = (x*m) @ W
  e   = leakyrelu(e_l[i] + e_r[j]),  e_l = h@a_l, e_r = h@a_r
  p   = adj*mask_j ? exp(e) : 0        (softmax numerator; exp(-1e4) == 0)
  out = LN(((p @ h) / rowsum(p) + x*m) * m) * gamma + beta

Device layout trick: the host feeds adj TRANSPOSED as bf16 {0,1} so the
[N,N] pipeline runs in [j-partition, i-free] orientation and the big
matmul p@h needs no on-chip transpose (contraction over j = partitions).
mask_j is folded additively into e_r (e_r - 1e4*(1-m_j)); mask_i is not
needed (masked rows are zeroed by the final *m).
"""

import os
import sys

import numpy as np

if "/opt/trn_rl_repo" not in sys.path:
    sys.path.insert(0, "/opt/trn_rl_repo")

B, N, D = 8, 2048, 128
NB = N // 128
ALPHA = 0.2
EPS = 1e-5
NCORES = 8

# Perf knobs: which j-blocks compute leakyrelu on DVE (3-op max trick)
# instead of ScalarE Lrelu, and which do the adj-mask multiply on GPSIMD.
DVE_LRELU_BLOCKS = frozenset({0, 3, 6, 9, 12, 15})
GPSIMD_MASK_BLOCKS = frozenset()

_PROG_CACHE = {}
RACE_DETECT = True  # sim_check disables: tail sem-decrements trip the sim's race detector
SEM_CLEAR_MODE = "skip"  # tail sem reset unnecessary (runtime resets between executions);
# "dec" variant crashes the device, "skip" verified correct across reruns
LAST_EXEC_TIME_NS = None
LAST_MEAN_EXEC_TIME_NS = None


def _patch_sem_clear():
    """This environment's walrus rejects EVENT_SEMAPHORE_RANGE_CLEAR
    ("ISA wrong length" — ISA table skew vs the repo).  Replace Tile's
    tail range-clear with per-semaphore decrements of each semaphore's
    statically-known net increment, which is equivalent for a
    deterministic program (each execution starts from the cleared
    state the previous one restored).
    """
    import bass_rust
    import concourse.bass as bass

    if getattr(bass.BassEngine, "_gat_sem_clear_patched", False):
        return

    def sem_clear(self, sem):
        if SEM_CLEAR_MODE == "skip":
            return None
        if not isinstance(sem, range):
            sem = range(sem.num, sem.num + 1)
        net = {s: 0 for s in sem}
        for b in self.bass.m.functions[0].blocks:
            for inst in b.instructions:
                si = inst.sync_info
                if si is None or not si.on_update:
                    continue
                for u in si.on_update:
                    if u.id in net:
                        if u.update_mode in ("sem-add-imm", "sem-inc"):
                            net[u.id] += u.update_value if u.update_value is not None else 1
                        elif u.update_mode in ("sem-dec",):
                            net[u.id] -= u.update_value if u.update_value is not None else 1
                        else:
                            raise AssertionError(u.update_mode)
        last = None
        for s in sem:
            if net[s]:
                h = bass_rust.SemaphoreHandle(name=f"semdec_{s}", num=s)
                last = self.sem_inc(h, -net[s])
        return last

    bass.BassEngine.sem_clear = sem_clear
    bass.BassEngine._gat_sem_clear_patched = True


def _split_waits(nc, mybir, max_waits=1):
    """This walrus build allows only one semaphore-wait slot per
    instruction ("Too many sync wait commands").  Hoist extra waits onto
    standalone EventSemaphore carrier instructions placed immediately
    before the offender on the same engine; the engine sequencer
    executes them in order, so the dependency semantics are unchanged.
    """
    for f in nc.m.functions:
        for b in f.blocks:
            il = b.instructions
            k = 0
            while k < len(il):
                i = il[k]
                si = i.sync_info
                if si is not None and si.on_wait and len(si.on_wait) > max_waits:
                    waits = list(si.on_wait)
                    extra, keep = waits[:-max_waits], waits[-max_waits:]
                    for j, w in enumerate(extra):
                        ev = mybir.InstEventSemaphore(
                            name=f"{i.name}-wsplit{j}",
                            engine=i.engine,
                            debug=i.debug,
                            sync_info=mybir.SyncInfo(on_wait=[w], on_update=[]),
                        )
                        il.insert(k + j, ev)
                    k += len(extra)
                    i.sync_info = mybir.SyncInfo(
                        on_wait=keep, on_update=list(si.on_update or []))
                k += 1
    return nc


def _knobs():
    dve = os.environ.get("GAT_DVE_LRELU")
    gp = os.environ.get("GAT_GP_MASK")
    d = frozenset(int(x) for x in dve.split(",")) if dve else DVE_LRELU_BLOCKS
    g = frozenset(int(x) for x in gp.split(",")) if gp else GPSIMD_MASK_BLOCKS
    return d, g


def _build_program(apply_affine: bool):
    import concourse.bass as bass
    import concourse.tile as tile
    from concourse import mybir
    from concourse.masks import make_identity

    _patch_sem_clear()
    dve_lrelu, gp_mask = _knobs()

    fp32 = mybir.dt.float32
    bf16 = mybir.dt.bfloat16
    A = mybir.AluOpType
    F = mybir.ActivationFunctionType

    nc = bass.Bass(use_seq_codegen=True, detect_race_conditions=RACE_DETECT)

    x_in = nc.declare_dram_parameter("x", [N, D], fp32, isOutput=False)
    adjt = nc.declare_dram_parameter("adjt", [N, N], bf16, isOutput=False)
    maskf = nc.declare_dram_parameter("maskf", [N], fp32, isOutput=False)
    w_in = nc.declare_dram_parameter("w", [D, D], bf16, isOutput=False)
    al_in = nc.declare_dram_parameter("al", [D], bf16, isOutput=False)
    ar_in = nc.declare_dram_parameter("ar", [D], bf16, isOutput=False)
    if apply_affine:
        g_in = nc.declare_dram_parameter("gamma", [D], fp32, isOutput=False)
        b_in = nc.declare_dram_parameter("beta", [D], fp32, isOutput=False)
    out_d = nc.declare_dram_parameter("out", [N, D], fp32, isOutput=True)

    el_dram = nc.dram_tensor("el_scratch", [N], bf16)

    def bcast(ap, parts=128):
        """Partition-broadcast read AP (step 0 on the partition dim)."""
        return bass.AP(tensor=ap.tensor, offset=ap.offset, ap=[[0, parts]] + list(ap.ap))

    with tile.TileContext(nc) as tc:
        with tc.tile_pool(name="persist", bufs=1) as per:
            ident_bf = per.tile([128, 128], bf16)
            make_identity(nc, ident_bf)
            ident_f32 = per.tile([128, 128], fp32)
            make_identity(nc, ident_f32)
            ones_col = per.tile([128, 1], bf16)
            nc.vector.memset(ones_col, 1.0)
            eps_col = per.tile([128, 1], fp32)
            nc.vector.memset(eps_col, EPS)

            m_col = per.tile([128, NB], fp32)
            nc.sync.dma_start(out=m_col, in_=maskf[:].rearrange("(b p) -> p b", p=128))
            w_sb = per.tile([128, D], bf16)
            nc.sync.dma_start(out=w_sb, in_=w_in[:, :])
            al_bc = per.tile([128, D], bf16)
            nc.sync.dma_start(out=al_bc, in_=bcast(al_in[:]))
            ar_bc = per.tile([128, D], bf16)
            nc.sync.dma_start(out=ar_bc, in_=bcast(ar_in[:]))
            if apply_affine:
                g_bc = per.tile([128, D], fp32)
                nc.sync.dma_start(out=g_bc, in_=bcast(g_in[:]))
                b_bc = per.tile([128, D], fp32)
                nc.sync.dma_start(out=b_bc, in_=bcast(b_in[:]))

            x_tiles = [per.tile([128, D], fp32, name=f"xt{i}", tag=f"x{i}") for i in range(NB)]
            adj_tiles = [per.tile([128, N], bf16, name=f"adjt{i}", tag=f"adj{i}") for i in range(NB)]
            xm_all = per.tile([128, NB, D], fp32)     # x*m, f32 (residual)
            xmT_all = per.tile([128, NB, D], bf16)    # (x*m)^T blocks
            h_all = per.tile([128, NB, D], bf16)      # h blocks [node, d]
            el_col = per.tile([128, NB], fp32)
            er_col = per.tile([128, NB], fp32)
            er2_col = per.tile([128, NB], fp32)
            el_bc = per.tile([128, N], bf16)          # e_l broadcast over partitions
            z_all = per.tile([128, NB, D], fp32)      # pre-LN result
            o_tiles = [per.tile([128, D], fp32, name=f"ot{i}", tag=f"o{i}") for i in range(NB)]
            mv_all = per.tile([128, NB, 2], fp32)     # bn_aggr mean/var
            oT_sb = per.tile([128, N], fp32)          # (p@h)^T copy
            rs_sb = per.tile([1, N], fp32)            # rowsums

            # ---- preprocessing: xm, xm^T, h, e_l, e_r -------------------
            with (
                tc.tile_pool(name="pp", bufs=3) as pp,
                tc.tile_pool(name="pp_ps", bufs=2, space="PSUM") as pp_ps,
            ):
                for ib in range(NB):
                    x_t = x_tiles[ib]
                    nc.sync.dma_start(out=x_t, in_=x_in[ib * 128:(ib + 1) * 128, :])
                    nc.vector.tensor_scalar(
                        out=xm_all[:, ib, :], in0=x_t,
                        scalar1=m_col[:, ib:ib + 1], scalar2=None, op0=A.mult)
                    xm_bf = pp.tile([128, D], bf16, tag="xmbf")
                    nc.vector.tensor_copy(out=xm_bf, in_=xm_all[:, ib, :])
                    xmT_ps = pp_ps.tile([128, D], bf16, tag="xmT")
                    nc.tensor.transpose(xmT_ps, xm_bf, ident_bf)
                    nc.vector.tensor_copy(out=xmT_all[:, ib, :], in_=xmT_ps)
                for ib in range(NB):
                    h_ps = pp_ps.tile([128, D], fp32, tag="h")
                    nc.tensor.matmul(h_ps, lhsT=xmT_all[:, ib, :], rhs=w_sb,
                                     start=True, stop=True)
                    nc.vector.tensor_copy(out=h_all[:, ib, :], in_=h_ps)
                    hal = pp.tile([128, D], fp32, tag="hal")
                    nc.vector.tensor_tensor(out=hal, in0=h_all[:, ib, :],
                                            in1=al_bc, op=A.mult)
                    nc.vector.tensor_reduce(out=el_col[:, ib:ib + 1], in_=hal,
                                            axis=mybir.AxisListType.X,
                                            op=A.add)
                    har = pp.tile([128, D], fp32, tag="har")
                    nc.vector.tensor_tensor(out=har, in0=h_all[:, ib, :],
                                            in1=ar_bc, op=A.mult)
                    nc.vector.tensor_reduce(out=er_col[:, ib:ib + 1], in_=har,
                                            axis=mybir.AxisListType.X,
                                            op=A.add)

                # e_r2 = e_r + 1e4*m - 1e4   (mask_j folded additively)
                tmp_col = pp.tile([128, NB], fp32, tag="tmpc")
                nc.vector.tensor_scalar(out=tmp_col, in0=m_col,
                                        scalar1=1e4, scalar2=-1e4,
                                        op0=A.mult, op1=A.add)
                nc.vector.tensor_tensor(out=er2_col, in0=er_col, in1=tmp_col,
                                        op=A.add)

                # e_l column -> row (PE transpose) -> DRAM -> broadcast tile
                el_bf_col = pp.tile([128, NB], bf16, tag="elbf")
                nc.vector.tensor_copy(out=el_bf_col, in_=el_col)
                elT_ps = pp_ps.tile([NB, 128], bf16, tag="elT")
                nc.tensor.transpose(elT_ps, el_bf_col, ident_bf)
                elT_sb = pp.tile([NB, 128], bf16, tag="elTs")
                nc.vector.tensor_copy(out=elT_sb, in_=elT_ps)
                nc.gpsimd.dma_start(out=el_dram[:].rearrange("(b q) -> b q", q=128),
                                    in_=elT_sb)
                nc.gpsimd.dma_start(out=el_bc, in_=bcast(el_dram[:]))

            # ---- main loop over j-blocks --------------------------------
            with (
                tc.tile_pool(name="mm_ps", bufs=1, space="PSUM") as mm_ps_pool,
                tc.tile_pool(name="rs_ps", bufs=1, space="PSUM") as rs_ps_pool,
                tc.tile_pool(name="blk", bufs=4) as blk,
                tc.tile_pool(name="ublk", bufs=4) as ublk,
            ):
                oT_ps = mm_ps_pool.tile([128, N], fp32)
                rs_ps = rs_ps_pool.tile([1, N], fp32)
                for jb in range(NB):
                    adj_t = adj_tiles[jb]
                    nc.sync.dma_start(out=adj_t,
                                      in_=adjt[jb * 128:(jb + 1) * 128, :])
                    er2_s = er2_col[:, jb:jb + 1]
                    u = ublk.tile([128, N], bf16, tag="u")
                    if jb in dve_lrelu:
                        t2 = ublk.tile([128, N], bf16, tag="t2")
                        nc.vector.tensor_scalar(out=t2, in0=el_bc, scalar1=er2_s,
                                                scalar2=None, op0=A.add)
                        ta = ublk.tile([128, N], bf16, tag="ta")
                        nc.vector.tensor_scalar(out=ta, in0=t2, scalar1=ALPHA,
                                                scalar2=None, op0=A.mult)
                        nc.vector.tensor_tensor(out=u, in0=t2, in1=ta, op=A.max)
                    else:
                        nc.scalar.activation(out=u, in_=el_bc, func=F.Lrelu,
                                             bias=er2_s, scale=1.0, alpha=ALPHA)
                    pexp = ublk.tile([128, N], bf16, tag="pexp")
                    nc.scalar.activation(out=pexp, in_=u, func=F.Exp)
                    pm = blk.tile([128, N], bf16, tag="pm")
                    eng = nc.gpsimd if jb in gp_mask else nc.vector
                    eng.tensor_tensor(out=pm, in0=pexp, in1=adj_t, op=A.mult)

                    st, sp = (jb == 0), (jb == NB - 1)
                    for s in range(4):
                        nc.tensor.matmul(oT_ps[:, s * 512:(s + 1) * 512],
                                         lhsT=h_all[:, jb, :],
                                         rhs=pm[:, s * 512:(s + 1) * 512],
                                         start=st, stop=sp)
                    for s in range(4):
                        nc.tensor.matmul(rs_ps[:, s * 512:(s + 1) * 512],
                                         lhsT=ones_col,
                                         rhs=pm[:, s * 512:(s + 1) * 512],
                                         start=st, stop=sp)

                nc.vector.tensor_copy(out=rs_sb, in_=rs_ps)
                nc.scalar.copy(out=oT_sb, in_=oT_ps)

            # ---- epilogue: normalize, residual, layernorm ---------------
            with (
                tc.tile_pool(name="ep", bufs=4) as ep,
                tc.tile_pool(name="ep_ps", bufs=2, space="PSUM") as ep_ps,
            ):
                # rowsum row [1,N] -> col [128,NB] via bounce + PE transpose
                rsT = ep.tile([NB, 128], fp32, tag="rsT")
                nc.gpsimd.dma_start(out=rsT,
                                    in_=rs_sb.rearrange("o (b q) -> o b q", q=128))
                rsc_ps = ep_ps.tile([128, NB], fp32, tag="rsc")
                nc.tensor.transpose(rsc_ps, rsT, ident_f32[:NB, :NB])
                r_col = ep.tile([128, NB], fp32, tag="rcol")
                nc.vector.reciprocal(out=r_col, in_=rsc_ps)
                rm_col = ep.tile([128, NB], fp32, tag="rmcol")
                nc.vector.tensor_tensor(out=rm_col, in0=r_col, in1=m_col,
                                        op=A.mult)

                for ib in range(NB):
                    tr_ps = ep_ps.tile([128, 128], fp32, tag="tr")
                    nc.tensor.transpose(tr_ps, oT_sb[:, ib * 128:(ib + 1) * 128],
                                        ident_f32)
                    z1 = ep.tile([128, 128], fp32, tag="z1")
                    nc.vector.tensor_scalar(out=z1, in0=tr_ps,
                                            scalar1=rm_col[:, ib:ib + 1],
                                            scalar2=None, op0=A.mult)
                    nc.vector.tensor_tensor(out=z_all[:, ib, :], in0=z1,
                                            in1=xm_all[:, ib, :], op=A.add)
                    st6 = ep.tile([128, 6], fp32, tag="st6")
                    nc.vector.bn_stats(out=st6, in_=z_all[:, ib, :])
                    nc.vector.bn_aggr(out=mv_all[:, ib, :], in_=st6)

                # rstd = exp(-0.5*ln(var+eps)) : stays in the exp/ln table set
                var_v = mv_all[:, :, 1:2].rearrange("p b o -> p (b o)")
                lnv = ep.tile([128, NB], fp32, tag="lnv")
                nc.scalar.activation(out=lnv, in_=var_v, func=F.Ln,
                                     bias=eps_col, scale=1.0)
                rstd = ep.tile([128, NB], fp32, tag="rstd")
                nc.scalar.activation(out=rstd, in_=lnv, func=F.Exp, scale=-0.5)

                for ib in range(NB):
                    o_t = o_tiles[ib]
                    nc.vector.tensor_scalar(
                        out=o_t, in0=z_all[:, ib, :],
                        scalar1=mv_all[:, ib, 0:1].rearrange("p o -> p o"),
                        scalar2=rstd[:, ib:ib + 1],
                        op0=A.subtract, op1=A.mult)
                    if apply_affine:
                        nc.vector.tensor_tensor(out=o_t, in0=o_t, in1=g_bc,
                                                op=A.mult)
                        nc.vector.tensor_tensor(out=o_t, in0=o_t, in1=b_bc,
                                                op=A.add)
                    nc.gpsimd.dma_start(out=out_d[ib * 128:(ib + 1) * 128, :],
                                        in_=o_t)
    return _split_waits(nc, mybir)


def _get_program(apply_affine: bool):
    key = (apply_affine, _knobs())
    if key not in _PROG_CACHE:
        _PROG_CACHE[key] = _build_program(apply_affine)
    return _PROG_CACHE[key]


def _prep_inputs(x, adj_bool, node_mask, W, a_l, a_r, gamma, beta, apply_affine):
    import ml_dtypes

    bf16 = ml_dtypes.bfloat16
    x = np.asarray(x, dtype=np.float32)
    adj_bool = np.asarray(adj_bool)
    node_mask = np.asarray(node_mask)
    w_bf = np.ascontiguousarray(np.asarray(W, dtype=np.float32).astype(bf16))
    al_bf = np.ascontiguousarray(np.asarray(a_l, dtype=np.float32).astype(bf16))
    ar_bf = np.ascontiguousarray(np.asarray(a_r, dtype=np.float32).astype(bf16))
    in_maps = []
    for b in range(NCORES):
        adjt = np.ascontiguousarray(adj_bool[b].T.astype(bf16))
        m = {
            "x": np.ascontiguousarray(x[b]),
            "adjt": adjt,
            "maskf": np.ascontiguousarray(node_mask[b].astype(np.float32)),
            "w": w_bf,
            "al": al_bf,
            "ar": ar_bf,
        }
        if apply_affine:
            m["gamma"] = np.ascontiguousarray(np.asarray(gamma, np.float32))
            m["beta"] = np.ascontiguousarray(np.asarray(beta, np.float32))
        in_maps.append(m)
    return in_maps


def kernel(x, adj_bool, node_mask, W, a_l, a_r, gamma, beta):
    global LAST_EXEC_TIME_NS, LAST_MEAN_EXEC_TIME_NS
    from concourse.bass_utils import run_bass_kernel_spmd

    gamma_np = np.asarray(gamma, dtype=np.float32)
    beta_np = np.asarray(beta, dtype=np.float32)
    apply_affine = not (np.all(gamma_np == 1.0) and np.all(beta_np == 0.0))

    nc = _get_program(apply_affine)
    in_maps = _prep_inputs(x, adj_bool, node_mask, W, a_l, a_r,
                           gamma_np, beta_np, apply_affine)
    trace = bool(int(os.environ.get("GAT_TRACE", "0")))
    res = run_bass_kernel_spmd(nc, in_maps, list(range(NCORES)), trace=trace)
    LAST_EXEC_TIME_NS = res.exec_time_ns
    LAST_MEAN_EXEC_TIME_NS = res.mean_exec_time_ns
    out = np.stack([np.asarray(r["out"], dtype=np.float32) for r in res.results])
    return out



# revision 8
# speedup vs baseline: 1.2667x; 1.2667x over previous
"""GAT layer (gnn_message_passing) Trainium2 Bass kernel, v2.

Per-core work (data-parallel over batch B=8, one graph per NeuronCore):
  h   = (x*m) @ W
  e   = leakyrelu(e_l[i] + e_r[j]),  e_l = h@a_l, e_r = h@a_r
  attn= softmax_j(adj&mask ? e : -inf)
  out = LN((attn @ h + x*m) * m) * gamma + beta

Key algebraic restructure (vs v1 which ran Lrelu+Exp over the full [N,N]
on ScalarE): exp(lrelu(s)) with s = e_l[i]+e_r[j] factorizes as
  exp(0.2s) * max(exp(0.8s), 1)
and exp(0.8s) = El8[i]*Er8[j] is rank-1.  The pure-i factor exp(0.2 e_l[i])
cancels between softmax numerator and denominator and is dropped.  With
r[j] = m[j]*exp(0.2 e_r[j]) the numerator becomes
  q[j,i] = adj[i,j] * max(El8[i]*(Er8*r)[j], r[j])
so the only [N,N] elementwise work per 128-row j-block is:
  1. one DVE tensor_scalar (mult+max, 4x bf16)    w_pre
  2. a gpsimd DMA that casts adj (fp8e5 {0,-32768}) to bf16 and
     ADD-accumulates it into w_pre in flight (CCE add)
  3. one relu (DVE tensor_scalar max-0 or ScalarE Relu, knob-split)
exp() runs only on [128,16] column vectors.  Rowsums use four col-tiled
M=1 matmuls (tile_position) accumulating alongside the main h^T@w matmul.
LayerNorm mean comes free from a fused tensor_tensor_reduce (residual add
+ row sum); variance via ScalarE Square with accum_out.
"""

import os
import sys

import numpy as np

if "/opt/trn_rl_repo" not in sys.path:
    sys.path.insert(0, "/opt/trn_rl_repo")

B, N, D = 8, 2048, 128
NB = N // 128
ALPHA = 0.2
EPS = 1e-5
NCORES = 8
ADJ_BIG = 32768.0  # exactly representable in fp8e5m2 and bf16

# Perf knob: which j-blocks run the post-DMA relu on ScalarE (rest on DVE).
SCALAR_RELU_BLOCKS = frozenset({1, 4, 7, 9, 12, 15})

_PROG_CACHE = {}
RACE_DETECT = True
SEM_CLEAR_MODE = "skip"  # tail sem reset unnecessary (runtime resets between executions)
LAST_EXEC_TIME_NS = None
LAST_MEAN_EXEC_TIME_NS = None


def _patch_sem_clear():
    """This environment's walrus rejects EVENT_SEMAPHORE_RANGE_CLEAR
    ("ISA wrong length" — ISA table skew vs the repo).  Tile's tail
    range-clear is unnecessary here (runtime resets between executions),
    so skip it.
    """
    import bass_rust
    import concourse.bass as bass

    if getattr(bass.BassEngine, "_gat_sem_clear_patched", False):
        return

    def sem_clear(self, sem):
        if SEM_CLEAR_MODE == "skip":
            return None
        if not isinstance(sem, range):
            sem = range(sem.num, sem.num + 1)
        net = {s: 0 for s in sem}
        for b in self.bass.m.functions[0].blocks:
            for inst in b.instructions:
                si = inst.sync_info
                if si is None or not si.on_update:
                    continue
                for u in si.on_update:
                    if u.id in net:
                        if u.update_mode in ("sem-add-imm", "sem-inc"):
                            net[u.id] += u.update_value if u.update_value is not None else 1
                        elif u.update_mode in ("sem-dec",):
                            net[u.id] -= u.update_value if u.update_value is not None else 1
                        else:
                            raise AssertionError(u.update_mode)
        last = None
        for s in sem:
            if net[s]:
                h = bass_rust.SemaphoreHandle(name=f"semdec_{s}", num=s)
                last = self.sem_inc(h, -net[s])
        return last

    bass.BassEngine.sem_clear = sem_clear
    bass.BassEngine._gat_sem_clear_patched = True


def _split_waits(nc, mybir, max_waits=1):
    """This walrus build allows only one semaphore-wait slot per
    instruction ("Too many sync wait commands").  Hoist extra waits onto
    standalone EventSemaphore carrier instructions placed immediately
    before the offender on the same engine; the engine sequencer
    executes them in order, so the dependency semantics are unchanged.
    """
    for f in nc.m.functions:
        for b in f.blocks:
            il = b.instructions
            k = 0
            while k < len(il):
                i = il[k]
                si = i.sync_info
                if si is not None and si.on_wait and len(si.on_wait) > max_waits:
                    waits = list(si.on_wait)
                    extra, keep = waits[:-max_waits], waits[-max_waits:]
                    for j, w in enumerate(extra):
                        ev = mybir.InstEventSemaphore(
                            name=f"{i.name}-wsplit{j}",
                            engine=i.engine,
                            debug=i.debug,
                            sync_info=mybir.SyncInfo(on_wait=[w], on_update=[]),
                        )
                        il.insert(k + j, ev)
                    k += len(extra)
                    i.sync_info = mybir.SyncInfo(
                        on_wait=keep, on_update=list(si.on_update or []))
                k += 1
    return nc


def _knobs():
    sr = os.environ.get("GAT_SCALAR_RELU")
    s = (frozenset(int(x) for x in sr.split(",") if x != "")
         if sr is not None else SCALAR_RELU_BLOCKS)
    return s


def _build_program(apply_affine: bool):
    import concourse.bass as bass
    import concourse.tile as tile
    from concourse import mybir
    from concourse.masks import make_identity

    _patch_sem_clear()
    scalar_relu = _knobs()

    fp32 = mybir.dt.float32
    bf16 = mybir.dt.bfloat16
    fp8e5 = mybir.dt.float8e5
    A = mybir.AluOpType
    F = mybir.ActivationFunctionType

    nc = bass.Bass(use_seq_codegen=True, detect_race_conditions=RACE_DETECT)

    x_in = nc.declare_dram_parameter("x", [N, D], fp32, isOutput=False)
    xt_in = nc.declare_dram_parameter("xt", [D, N], bf16, isOutput=False)
    adjadd = nc.declare_dram_parameter("adjadd", [N, N], fp8e5, isOutput=False)
    maskf = nc.declare_dram_parameter("maskf", [N], fp32, isOutput=False)
    wfull_in = nc.declare_dram_parameter("wfull", [D, D + 2], bf16, isOutput=False)
    if apply_affine:
        g_in = nc.declare_dram_parameter("gamma", [D], fp32, isOutput=False)
        b_in = nc.declare_dram_parameter("beta", [D], fp32, isOutput=False)
    out_d = nc.declare_dram_parameter("out", [N, D], fp32, isOutput=True)

    el_dram = nc.dram_tensor("el8_scratch", [N], bf16)

    def bcast(ap, parts=128):
        """Partition-broadcast read AP (step 0 on the partition dim)."""
        return bass.AP(tensor=ap.tensor, offset=ap.offset, ap=[[0, parts]] + list(ap.ap))

    with tile.TileContext(nc) as tc:
        with tc.tile_pool(name="persist", bufs=1) as per:
            eps_col = per.tile([128, 1], fp32)
            nc.vector.memset(eps_col, EPS)
            warm = per.tile([128, 1], fp32)
            # fires the exp/ln ACT table load early, off the critical path
            nc.scalar.activation(out=warm, in_=eps_col, func=F.Exp)

            ident_f32 = per.tile([128, 128], fp32)
            make_identity(nc, ident_f32)
            ident_bf = per.tile([128, 128], bf16)
            make_identity(nc, ident_bf)
            ones_col = per.tile([128, 1], bf16)
            nc.vector.memset(ones_col, 1.0)

            m_col = per.tile([128, NB], fp32)
            nc.sync.dma_start(out=m_col, in_=maskf[:].rearrange("(b p) -> p b", p=128))
            w_full = per.tile([128, D + 2], bf16)
            nc.sync.dma_start(out=w_full, in_=wfull_in[:, :])
            xT_sb = per.tile([128, N], bf16)
            nc.sync.dma_start(out=xT_sb, in_=xt_in[:, :])
            if apply_affine:
                g_bc = per.tile([128, D], fp32)
                nc.sync.dma_start(out=g_bc, in_=bcast(g_in[:]))
                b_bc = per.tile([128, D], fp32)
                nc.sync.dma_start(out=b_bc, in_=bcast(b_in[:]))

            xm_all = per.tile([128, NB, D], fp32)      # x*m rows (residual)
            hel_all = per.tile([128, NB, D + 2], bf16)  # h | e_l | e_r per block
            el8_bc = per.tile([128, N], bf16)          # exp(0.8 e_l) row-bcast
            er8r_col = per.tile([128, NB], fp32)       # exp(0.8 e_r) * r
            r_col = per.tile([128, NB], fp32)          # m * exp(0.2 e_r)
            z_all = per.tile([128, NB, D], fp32)       # pre-LN rows
            zs_col = per.tile([128, NB], fp32)         # sum_d z1 (attn part)
            xs_col = per.tile([128, NB], fp32)         # sum_d x*m
            z2s_col = per.tile([128, NB], fp32)        # sum_d z^2
            oT_sb = per.tile([128, N], fp32)           # (q@h)^T copy
            rs_sb = per.tile([128, 512], fp32)         # rowsum strips
            rsT = per.tile([NB, 128], fp32)
            rm_col = per.tile([128, NB], fp32)
            rstd_col = per.tile([128, NB], fp32)
            nmr_col = per.tile([128, NB], fp32)        # -mu*rstd

            # ---- preprocessing: xm, h, e_l, e_r, exps, el8 broadcast ----
            with (
                tc.tile_pool(name="pp", bufs=3) as pp,
                tc.tile_pool(name="pp_ps", bufs=2, space="PSUM") as pp_ps,
            ):
                for ib in range(NB):
                    x_t = pp.tile([128, D], fp32, tag="x")
                    nc.sync.dma_start(out=x_t, in_=x_in[ib * 128:(ib + 1) * 128, :])
                    nc.vector.tensor_scalar(
                        out=xm_all[:, ib, :], in0=x_t,
                        scalar1=m_col[:, ib:ib + 1], scalar2=0.0,
                        op0=A.mult, op1=A.add,
                        accum_out=xs_col[:, ib:ib + 1])
                    hr_ps = pp_ps.tile([128, D + 2], fp32, tag="hr")
                    nc.tensor.matmul(hr_ps,
                                     lhsT=xT_sb[:, ib * 128:(ib + 1) * 128],
                                     rhs=w_full, start=True, stop=True)
                    nc.vector.tensor_copy(out=hel_all[:, ib, :], in_=hr_ps)

                el_v = hel_all[:, :, D]        # [128, NB] strided bf16
                er_v = hel_all[:, :, D + 1]
                el8_col = pp.tile([128, NB], bf16, tag="el8")
                nc.scalar.activation(out=el8_col, in_=el_v, func=F.Exp, scale=0.8)
                er8_col = pp.tile([128, NB], fp32, tag="er8")
                nc.scalar.activation(out=er8_col, in_=er_v, func=F.Exp, scale=0.8)
                er2_col = pp.tile([128, NB], fp32, tag="er2")
                nc.scalar.activation(out=er2_col, in_=er_v, func=F.Exp, scale=0.2)
                nc.vector.tensor_tensor(out=r_col, in0=er2_col, in1=m_col,
                                        op=A.mult)
                nc.vector.tensor_tensor(out=er8r_col, in0=er8_col, in1=r_col,
                                        op=A.mult)

                # el8 column -> row (PE transpose) -> DRAM -> broadcast tile
                elT_ps = pp_ps.tile([NB, 128], bf16, tag="elT")
                nc.tensor.transpose(elT_ps, el8_col, ident_bf)
                elT_sb = pp.tile([NB, 128], bf16, tag="elTs")
                nc.vector.tensor_copy(out=elT_sb, in_=elT_ps)
                nc.gpsimd.dma_start(out=el_dram[:].rearrange("(b q) -> b q", q=128),
                                    in_=elT_sb)
                nc.gpsimd.dma_start(out=el8_bc, in_=bcast(el_dram[:]))

            # ---- main loop over j-blocks --------------------------------
            with (
                tc.tile_pool(name="mm_ps", bufs=1, space="PSUM") as mm_ps_pool,
                tc.tile_pool(name="rs_ps", bufs=1, space="PSUM") as rs_ps_pool,
                tc.tile_pool(name="wp", bufs=5) as wp,
            ):
                oT_ps = mm_ps_pool.tile([128, N], fp32)
                rs_ps = rs_ps_pool.tile([128, 512], fp32)
                for jb in range(NB):
                    w_t = wp.tile([128, N], bf16, tag="w")
                    nc.vector.tensor_scalar(
                        out=w_t, in0=el8_bc,
                        scalar1=er8r_col[:, jb:jb + 1],
                        scalar2=r_col[:, jb:jb + 1],
                        op0=A.mult, op1=A.max)
                    nc.gpsimd.dma_start(
                        out=w_t, in_=adjadd[jb * 128:(jb + 1) * 128, :],
                        accum_op=A.add)
                    if jb in scalar_relu:
                        nc.scalar.activation(out=w_t, in_=w_t, func=F.Relu)
                    else:
                        nc.vector.tensor_scalar(out=w_t, in0=w_t, scalar1=0.0,
                                                scalar2=None, op0=A.max)

                    st, sp = (jb == 0), (jb == NB - 1)
                    for s in range(4):
                        nc.tensor.matmul(oT_ps[:, s * 512:(s + 1) * 512],
                                         lhsT=hel_all[:, jb, 0:D],
                                         rhs=w_t[:, s * 512:(s + 1) * 512],
                                         start=st, stop=sp)
                        nc.tensor.matmul(rs_ps[32 * s:32 * s + 1, :],
                                         lhsT=ones_col,
                                         rhs=w_t[:, s * 512:(s + 1) * 512],
                                         start=st, stop=sp,
                                         tile_position=(0, 32 * s))

                nc.scalar.copy(out=oT_sb[:, 0:1024], in_=oT_ps[:, 0:1024])
                nc.vector.tensor_copy(out=oT_sb[:, 1024:2048],
                                      in_=oT_ps[:, 1024:2048])
                nc.vector.tensor_copy(out=rs_sb, in_=rs_ps)

            # ---- epilogue: normalize, residual, layernorm ---------------
            with (
                tc.tile_pool(name="ep", bufs=4) as ep,
                tc.tile_pool(name="ep_ps", bufs=3, space="PSUM") as ep_ps,
            ):
                # rowsum strips [1,512]@part 32s -> col layout [128, NB]
                for s in range(4):
                    nc.gpsimd.dma_start(
                        out=rsT[4 * s:4 * s + 4, :],
                        in_=rs_sb[32 * s:32 * s + 1, :].rearrange(
                            "o (b q) -> o b q", q=128))
                rsc_ps = ep_ps.tile([128, NB], fp32, tag="rsc")
                nc.tensor.transpose(rsc_ps, rsT, ident_f32[:NB, :NB])
                rc_col = ep.tile([128, NB], fp32, tag="rcol")
                nc.vector.reciprocal(out=rc_col, in_=rsc_ps)
                nc.vector.tensor_tensor(out=rm_col, in0=rc_col, in1=m_col,
                                        op=A.mult)

                for ib in range(NB):
                    tr_ps = ep_ps.tile([128, 128], fp32, tag="tr")
                    nc.tensor.transpose(tr_ps, oT_sb[:, ib * 128:(ib + 1) * 128],
                                        ident_f32)
                    z1 = ep.tile([128, 128], fp32, tag="z1")
                    nc.scalar.activation(out=z1, in_=tr_ps, func=F.Copy,
                                         scale=rm_col[:, ib:ib + 1],
                                         accum_out=zs_col[:, ib:ib + 1])
                    nc.vector.tensor_tensor(
                        out=z_all[:, ib, :], in0=z1, in1=xm_all[:, ib, :],
                        op=A.add)
                    sq = ep.tile([128, 128], fp32, tag="sq")
                    nc.scalar.activation(out=sq, in_=z_all[:, ib, :],
                                         func=F.Square,
                                         accum_out=z2s_col[:, ib:ib + 1])

                # mu = (zs+xs)/D ; var = z2s/D - mu^2 ; rstd = exp(-.5 ln(var+eps))
                mu_col = ep.tile([128, NB], fp32, tag="mu")
                nc.vector.tensor_tensor(out=mu_col, in0=zs_col, in1=xs_col,
                                        op=A.add)
                nc.vector.tensor_scalar(out=mu_col, in0=mu_col,
                                        scalar1=1.0 / D, scalar2=None,
                                        op0=A.mult)
                mu2_col = ep.tile([128, NB], fp32, tag="mu2")
                nc.vector.tensor_tensor(out=mu2_col, in0=mu_col, in1=mu_col,
                                        op=A.mult)
                var_col = ep.tile([128, NB], fp32, tag="var")
                nc.vector.tensor_scalar(out=var_col, in0=z2s_col,
                                        scalar1=1.0 / D, scalar2=None,
                                        op0=A.mult)
                nc.vector.tensor_tensor(out=var_col, in0=var_col, in1=mu2_col,
                                        op=A.subtract)
                lnv_col = ep.tile([128, NB], fp32, tag="lnv")
                nc.scalar.activation(out=lnv_col, in_=var_col, func=F.Ln,
                                     bias=eps_col, scale=1.0)
                nc.scalar.activation(out=rstd_col, in_=lnv_col, func=F.Exp,
                                     scale=-0.5)
                nc.vector.tensor_tensor(out=nmr_col, in0=mu_col, in1=rstd_col,
                                        op=A.mult)
                nc.vector.tensor_scalar(out=nmr_col, in0=nmr_col, scalar1=-1.0,
                                        scalar2=None, op0=A.mult)

                for ib in range(NB):
                    o_t = ep.tile([128, D], fp32, tag="o")
                    nc.scalar.activation(out=o_t, in_=z_all[:, ib, :],
                                         func=F.Identity,
                                         bias=nmr_col[:, ib:ib + 1],
                                         scale=rstd_col[:, ib:ib + 1])
                    if apply_affine:
                        nc.vector.tensor_tensor(out=o_t, in0=o_t, in1=g_bc,
                                                op=A.mult)
                        nc.vector.tensor_tensor(out=o_t, in0=o_t, in1=b_bc,
                                                op=A.add)
                    nc.sync.dma_start(out=out_d[ib * 128:(ib + 1) * 128, :],
                                      in_=o_t)
    return _split_waits(nc, mybir)


def _get_program(apply_affine: bool):
    key = (apply_affine, _knobs())
    if key not in _PROG_CACHE:
        _PROG_CACHE[key] = _build_program(apply_affine)
    return _PROG_CACHE[key]


def _prep_inputs(x, adj_bool, node_mask, W, a_l, a_r, gamma, beta, apply_affine):
    import ml_dtypes

    bf16 = ml_dtypes.bfloat16
    f8e5 = ml_dtypes.float8_e5m2
    x = np.asarray(x, dtype=np.float32)
    adj_bool = np.asarray(adj_bool)
    node_mask = np.asarray(node_mask)
    W32 = np.asarray(W, dtype=np.float32)
    wal = W32 @ np.asarray(a_l, np.float32)
    war = W32 @ np.asarray(a_r, np.float32)
    wfull = np.concatenate([W32, wal[:, None], war[:, None]], axis=1)
    wfull_bf = np.ascontiguousarray(wfull.astype(bf16))
    in_maps = []
    for b in range(NCORES):
        adjadd = np.where(adj_bool[b].T > 0, np.float32(0.0),
                          np.float32(-ADJ_BIG)).astype(f8e5)
        m = {
            "x": np.ascontiguousarray(x[b]),
            "xt": np.ascontiguousarray(x[b].T.astype(bf16)),
            "adjadd": np.ascontiguousarray(adjadd),
            "maskf": np.ascontiguousarray(node_mask[b].astype(np.float32)),
            "wfull": wfull_bf,
        }
        if apply_affine:
            m["gamma"] = np.ascontiguousarray(np.asarray(gamma, np.float32))
            m["beta"] = np.ascontiguousarray(np.asarray(beta, np.float32))
        in_maps.append(m)
    return in_maps


def kernel(x, adj_bool, node_mask, W, a_l, a_r, gamma, beta):
    global LAST_EXEC_TIME_NS, LAST_MEAN_EXEC_TIME_NS
    from concourse.bass_utils import run_bass_kernel_spmd

    gamma_np = np.asarray(gamma, dtype=np.float32)
    beta_np = np.asarray(beta, dtype=np.float32)
    apply_affine = not (np.all(gamma_np == 1.0) and np.all(beta_np == 0.0))

    nc = _get_program(apply_affine)
    in_maps = _prep_inputs(x, adj_bool, node_mask, W, a_l, a_r,
                           gamma_np, beta_np, apply_affine)
    trace = bool(int(os.environ.get("GAT_TRACE", "0")))
    res = run_bass_kernel_spmd(nc, in_maps, list(range(NCORES)), trace=trace)
    LAST_EXEC_TIME_NS = res.exec_time_ns
    LAST_MEAN_EXEC_TIME_NS = res.mean_exec_time_ns
    out = np.stack([np.asarray(r["out"], dtype=np.float32) for r in res.results])
    return out


# revision 9
# speedup vs baseline: 1.3294x; 1.0495x over previous
"""GAT layer (gnn_message_passing) Trainium2 Bass kernel, v3.

Per-core work (data-parallel over batch B=8, one graph per NeuronCore):
  h   = (x*m) @ W
  e   = leakyrelu(e_l[i] + e_r[j]),  e_l = h@a_l, e_r = h@a_r
  attn= softmax_j(adj&mask ? e : -inf)
  out = LN((attn @ h + x*m) * m) * gamma + beta

Algebra: exp(lrelu(s)) with s = e_l[i]+e_r[j] factorizes as
exp(0.2s)*max(exp(0.8s),1); exp(0.8s) = El8[i]*Er8[j] is rank-1 and the
pure-i factor exp(0.2 e_l[i]) cancels in the softmax.  With
r[j] = m[j]*exp(0.2 e_r[j]) the numerator is
  q[j,i] = adj[i,j] * max(El8[i]*(Er8*r)[j], r[j])
so the per-j-block [128,2048] elementwise work is one DVE tensor_scalar
(mult+max against the El8 broadcast row) and one tensor_tensor multiply
with the prefetched {0,1} adjacency (fp8e5 in HBM, cast to bf16 by a
SWDGE dma).  exp() runs only on [128,16] columns.

Matmul orientation: w blocks are the STATIONARY operand, rhs = [ones|h],
so each (jb,ib) matmul accumulates out[i,d] AND the softmax denominator
(ones column) in one stream; the output lands row-major for the LN
epilogue (no transposes, no PSUM->SBUF bulk copy, no rowsum bounce).
LN sums come from ScalarE accum_out (Copy for mean, Square for var);
sum_d x rides the prep matmul as an extra ones column of wfull.
"""

import os
import sys

import numpy as np

if "/opt/trn_rl_repo" not in sys.path:
    sys.path.insert(0, "/opt/trn_rl_repo")

B, N, D = 8, 2048, 128
NB = N // 128
ALPHA = 0.2
EPS = 1e-5
NCORES = 8

# Perf knobs: blocks whose adjacency-mask multiply runs on GPSIMD
# instead of DVE, and blocks whose residual add runs on GPSIMD.
GP_TT_BLOCKS = frozenset()
GP_ZADD_BLOCKS = frozenset()

_PROG_CACHE = {}
RACE_DETECT = True
SEM_CLEAR_MODE = "skip"
LAST_EXEC_TIME_NS = None
LAST_MEAN_EXEC_TIME_NS = None


def _patch_sem_clear():
    """This environment's walrus rejects EVENT_SEMAPHORE_RANGE_CLEAR
    ("ISA wrong length" — ISA table skew vs the repo).  Tile's tail
    range-clear is unnecessary here (runtime resets between executions),
    so skip it.
    """
    import bass_rust
    import concourse.bass as bass

    if getattr(bass.BassEngine, "_gat_sem_clear_patched", False):
        return

    def sem_clear(self, sem):
        if SEM_CLEAR_MODE == "skip":
            return None
        return None

    bass.BassEngine.sem_clear = sem_clear
    bass.BassEngine._gat_sem_clear_patched = True


def _split_waits(nc, mybir, max_waits=1):
    """This walrus build allows only one semaphore-wait slot per
    instruction ("Too many sync wait commands").  Hoist extra waits onto
    standalone EventSemaphore carrier instructions placed immediately
    before the offender on the same engine; the engine sequencer
    executes them in order, so the dependency semantics are unchanged.
    """
    for f in nc.m.functions:
        for b in f.blocks:
            il = b.instructions
            k = 0
            while k < len(il):
                i = il[k]
                si = i.sync_info
                if si is not None and si.on_wait and len(si.on_wait) > max_waits:
                    waits = list(si.on_wait)
                    extra, keep = waits[:-max_waits], waits[-max_waits:]
                    for j, w in enumerate(extra):
                        ev = mybir.InstEventSemaphore(
                            name=f"{i.name}-wsplit{j}",
                            engine=i.engine,
                            debug=i.debug,
                            sync_info=mybir.SyncInfo(on_wait=[w], on_update=[]),
                        )
                        il.insert(k + j, ev)
                    k += len(extra)
                    i.sync_info = mybir.SyncInfo(
                        on_wait=keep, on_update=list(si.on_update or []))
                k += 1
    return nc


def _parse_blocks(env, default):
    v = os.environ.get(env)
    if v is None:
        return default
    return frozenset(int(x) for x in v.split(",") if x != "")


def _knobs():
    return (_parse_blocks("GAT_TT_G", GP_TT_BLOCKS),
            _parse_blocks("GAT_ZADD_G", GP_ZADD_BLOCKS))


def _build_program(apply_affine: bool):
    import concourse.bass as bass
    import concourse.tile as tile
    from concourse import mybir
    from concourse.masks import make_identity

    _patch_sem_clear()
    gp_tt, gp_zadd = _knobs()

    fp32 = mybir.dt.float32
    bf16 = mybir.dt.bfloat16
    fp8e5 = mybir.dt.float8e5
    A = mybir.AluOpType
    F = mybir.ActivationFunctionType

    nc = bass.Bass(use_seq_codegen=True, detect_race_conditions=RACE_DETECT)

    x_in = nc.declare_dram_parameter("x", [N, D], fp32, isOutput=False)
    xt_in = nc.declare_dram_parameter("xt", [D, N], bf16, isOutput=False)
    adj_in = nc.declare_dram_parameter("adj01", [N, N], fp8e5, isOutput=False)
    maskf = nc.declare_dram_parameter("maskf", [N], fp32, isOutput=False)
    # wfull = [W | W@a_l | W@a_r | ones]  ->  x@wfull = [h | e_l | e_r | sum_d x]
    wfull_in = nc.declare_dram_parameter("wfull", [D, D + 3], bf16, isOutput=False)
    if apply_affine:
        g_in = nc.declare_dram_parameter("gamma", [D], fp32, isOutput=False)
        b_in = nc.declare_dram_parameter("beta", [D], fp32, isOutput=False)
    out_d = nc.declare_dram_parameter("out", [N, D], fp32, isOutput=True)

    el_dram = nc.dram_tensor("el8_scratch", [N], bf16)

    def bcast(ap, parts=128):
        return bass.AP(tensor=ap.tensor, offset=ap.offset, ap=[[0, parts]] + list(ap.ap))

    with tile.TileContext(nc) as tc:
        with tc.tile_pool(name="persist", bufs=1) as per:
            eps_col = per.tile([128, 1], fp32)
            nc.vector.memset(eps_col, EPS)
            warm = per.tile([128, 1], fp32)
            nc.scalar.activation(out=warm, in_=eps_col, func=F.Exp)

            ident_bf = per.tile([128, 128], bf16)
            make_identity(nc, ident_bf)

            # adjacency prefetch: fp8e5 {0,1} in HBM -> bf16 tiles in SBUF
            adj_all = per.tile([128, NB, N], bf16)
            for g in range(4):
                nc.gpsimd.dma_start(
                    out=adj_all[:, 4 * g:4 * g + 4, :],
                    in_=adj_in[4 * g * 128:(4 * g + 4) * 128, :].rearrange(
                        "(c p) i -> p c i", p=128))

            m_col = per.tile([128, NB], fp32)
            nc.sync.dma_start(out=m_col, in_=maskf[:].rearrange("(b p) -> p b", p=128))
            w_full = per.tile([128, D + 3], bf16)
            nc.sync.dma_start(out=w_full, in_=wfull_in[:, :])
            xT_sb = per.tile([128, N], bf16)
            nc.sync.dma_start(out=xT_sb, in_=xt_in[:, :])
            if apply_affine:
                g_bc = per.tile([128, D], fp32)
                nc.sync.dma_start(out=g_bc, in_=bcast(g_in[:]))
                b_bc = per.tile([128, D], fp32)
                nc.sync.dma_start(out=b_bc, in_=bcast(b_in[:]))

            xm_all = per.tile([128, NB, D], fp32)       # x*m rows (residual)
            # hel layout per block: [ones | h(128) | e_l | e_r | sum_d x]
            hel_all = per.tile([128, NB, D + 4], bf16)
            nc.vector.memset(hel_all[:, :, 0], 1.0)
            el8_bc = per.tile([128, N], bf16)
            er8r_col = per.tile([128, NB], fp32)
            r_col = per.tile([128, NB], fp32)
            z_all = per.tile([128, NB, D], fp32)
            zs_col = per.tile([128, NB], fp32)          # sum_d z1 (attn part)
            z2s_col = per.tile([128, NB], fp32)         # sum_d z^2
            rm_col = per.tile([128, NB], fp32)
            rstd_col = per.tile([128, NB], fp32)
            nmr_col = per.tile([128, NB], fp32)

            # ---- prep: xm, h|el|er|xs, exps, el8 broadcast --------------
            with (
                tc.tile_pool(name="pp", bufs=3) as pp,
                tc.tile_pool(name="pp_ps", bufs=2, space="PSUM") as pp_ps,
            ):
                for ib in range(NB):
                    hr_ps = pp_ps.tile([128, D + 3], fp32, tag="hr")
                    nc.tensor.matmul(hr_ps,
                                     lhsT=xT_sb[:, ib * 128:(ib + 1) * 128],
                                     rhs=w_full, start=True, stop=True)
                    nc.vector.tensor_copy(out=hel_all[:, ib, 1:D + 4], in_=hr_ps)
                    x_t = pp.tile([128, D], fp32, tag="x")
                    nc.sync.dma_start(out=x_t, in_=x_in[ib * 128:(ib + 1) * 128, :])
                    nc.vector.tensor_scalar(
                        out=xm_all[:, ib, :], in0=x_t,
                        scalar1=m_col[:, ib:ib + 1], scalar2=None, op0=A.mult)

                el_v = hel_all[:, :, D + 1]     # [128, NB] strided bf16
                er_v = hel_all[:, :, D + 2]
                el8_col = pp.tile([128, NB], bf16, tag="el8")
                nc.scalar.activation(out=el8_col, in_=el_v, func=F.Exp, scale=0.8)
                er8_col = pp.tile([128, NB], fp32, tag="er8")
                nc.scalar.activation(out=er8_col, in_=er_v, func=F.Exp, scale=0.8)
                er2_col = pp.tile([128, NB], fp32, tag="er2")
                nc.scalar.activation(out=er2_col, in_=er_v, func=F.Exp, scale=0.2)
                nc.vector.tensor_tensor(out=r_col, in0=er2_col, in1=m_col,
                                        op=A.mult)
                nc.vector.tensor_tensor(out=er8r_col, in0=er8_col, in1=r_col,
                                        op=A.mult)

                elT_ps = pp_ps.tile([NB, 128], bf16, tag="elT")
                nc.tensor.transpose(elT_ps, el8_col, ident_bf)
                elT_sb = pp.tile([NB, 128], bf16, tag="elTs")
                nc.vector.tensor_copy(out=elT_sb, in_=elT_ps)
                nc.gpsimd.dma_start(out=el_dram[:].rearrange("(b q) -> b q", q=128),
                                    in_=elT_sb)
                nc.gpsimd.dma_start(out=el8_bc, in_=bcast(el_dram[:]))

            # ---- main: w blocks stationary, rhs=[ones|h] accumulates ----
            # out_all[:, ib, 0] = rowsum, [:, ib, 1:129] = (attn_num @ h)[i,d]
            with (
                tc.tile_pool(name="mm_ps", bufs=1, space="PSUM") as mm_ps_pool,
                tc.tile_pool(name="wp", bufs=4) as wp,
            ):
                out_all = mm_ps_pool.tile([128, NB, 256], fp32)
                for jb in range(NB):
                    w_t = wp.tile([128, N], bf16, tag="w")
                    nc.vector.tensor_scalar(
                        out=w_t, in0=el8_bc,
                        scalar1=er8r_col[:, jb:jb + 1],
                        scalar2=r_col[:, jb:jb + 1],
                        op0=A.mult, op1=A.max)
                    eng = nc.gpsimd if jb in gp_tt else nc.vector
                    eng.tensor_tensor(out=w_t, in0=w_t,
                                      in1=adj_all[:, jb, :], op=A.mult)
                    st, sp = (jb == 0), (jb == NB - 1)
                    for ib in range(NB):
                        nc.tensor.matmul(
                            out_all[:, ib, 0:129],
                            lhsT=w_t[:, ib * 128:(ib + 1) * 128],
                            rhs=hel_all[:, jb, 0:129],
                            start=st, stop=sp)

                # ---- epilogue: normalize, residual, layernorm -----------
                with tc.tile_pool(name="ep", bufs=4) as ep:
                    rs_sb = ep.tile([128, NB], fp32, tag="rs")
                    nc.vector.tensor_copy(out=rs_sb, in_=out_all[:, :, 0])
                    rc_col = ep.tile([128, NB], fp32, tag="rc")
                    nc.vector.reciprocal(out=rc_col, in_=rs_sb)
                    nc.vector.tensor_tensor(out=rm_col, in0=rc_col, in1=m_col,
                                            op=A.mult)

                    for ib in range(NB):
                        z1 = ep.tile([128, 128], fp32, tag="z1")
                        nc.scalar.activation(out=z1, in_=out_all[:, ib, 1:129],
                                             func=F.Copy,
                                             scale=rm_col[:, ib:ib + 1],
                                             accum_out=zs_col[:, ib:ib + 1])
                        zeng = nc.gpsimd if ib in gp_zadd else nc.vector
                        zeng.tensor_tensor(
                            out=z_all[:, ib, :], in0=z1, in1=xm_all[:, ib, :],
                            op=A.add)
                        sq = ep.tile([128, 128], fp32, tag="sq")
                        nc.scalar.activation(out=sq, in_=z_all[:, ib, :],
                                             func=F.Square,
                                             accum_out=z2s_col[:, ib:ib + 1])

                    # mu=(zs+xs*m)/D; var=z2s/D-mu^2; rstd=exp(-.5 ln(var+eps))
                    xs_v = hel_all[:, :, D + 3]
                    xsm_col = ep.tile([128, NB], fp32, tag="xsm")
                    nc.vector.tensor_tensor(out=xsm_col, in0=xs_v, in1=m_col,
                                            op=A.mult)
                    mu_col = ep.tile([128, NB], fp32, tag="mu")
                    nc.vector.tensor_tensor(out=mu_col, in0=zs_col, in1=xsm_col,
                                            op=A.add)
                    nc.vector.tensor_scalar(out=mu_col, in0=mu_col,
                                            scalar1=1.0 / D, scalar2=None,
                                            op0=A.mult)
                    mu2_col = ep.tile([128, NB], fp32, tag="mu2")
                    nc.vector.tensor_tensor(out=mu2_col, in0=mu_col, in1=mu_col,
                                            op=A.mult)
                    var_col = ep.tile([128, NB], fp32, tag="var")
                    nc.vector.tensor_scalar(out=var_col, in0=z2s_col,
                                            scalar1=1.0 / D, scalar2=None,
                                            op0=A.mult)
                    nc.vector.tensor_tensor(out=var_col, in0=var_col,
                                            in1=mu2_col, op=A.subtract)
                    lnv_col = ep.tile([128, NB], fp32, tag="lnv")
                    nc.scalar.activation(out=lnv_col, in_=var_col, func=F.Ln,
                                         bias=eps_col, scale=1.0)
                    nc.scalar.activation(out=rstd_col, in_=lnv_col, func=F.Exp,
                                         scale=-0.5)
                    nc.vector.tensor_tensor(out=nmr_col, in0=mu_col,
                                            in1=rstd_col, op=A.mult)
                    nc.vector.tensor_scalar(out=nmr_col, in0=nmr_col,
                                            scalar1=-1.0, scalar2=None,
                                            op0=A.mult)

                    for ib in range(NB):
                        o_t = ep.tile([128, D], fp32, tag="o")
                        nc.scalar.activation(out=o_t, in_=z_all[:, ib, :],
                                             func=F.Identity,
                                             bias=nmr_col[:, ib:ib + 1],
                                             scale=rstd_col[:, ib:ib + 1])
                        if apply_affine:
                            nc.vector.tensor_tensor(out=o_t, in0=o_t, in1=g_bc,
                                                    op=A.mult)
                            nc.vector.tensor_tensor(out=o_t, in0=o_t, in1=b_bc,
                                                    op=A.add)
                        nc.sync.dma_start(out=out_d[ib * 128:(ib + 1) * 128, :],
                                          in_=o_t)
    return _split_waits(nc, mybir)


def _get_program(apply_affine: bool):
    key = (apply_affine, _knobs())
    if key not in _PROG_CACHE:
        _PROG_CACHE[key] = _build_program(apply_affine)
    return _PROG_CACHE[key]


def _prep_inputs(x, adj_bool, node_mask, W, a_l, a_r, gamma, beta, apply_affine):
    import ml_dtypes

    bf16 = ml_dtypes.bfloat16
    f8e5 = ml_dtypes.float8_e5m2
    x = np.asarray(x, dtype=np.float32)
    adj_bool = np.asarray(adj_bool)
    node_mask = np.asarray(node_mask)
    W32 = np.asarray(W, dtype=np.float32)
    wal = W32 @ np.asarray(a_l, np.float32)
    war = W32 @ np.asarray(a_r, np.float32)
    ones = np.ones((D, 1), np.float32)
    wfull = np.concatenate([W32, wal[:, None], war[:, None], ones], axis=1)
    wfull_bf = np.ascontiguousarray(wfull.astype(bf16))
    in_maps = []
    for b in range(NCORES):
        adj01 = adj_bool[b].T.astype(np.float32).astype(f8e5)
        m = {
            "x": np.ascontiguousarray(x[b]),
            "xt": np.ascontiguousarray(x[b].T.astype(bf16)),
            "adj01": np.ascontiguousarray(adj01),
            "maskf": np.ascontiguousarray(node_mask[b].astype(np.float32)),
            "wfull": wfull_bf,
        }
        if apply_affine:
            m["gamma"] = np.ascontiguousarray(np.asarray(gamma, np.float32))
            m["beta"] = np.ascontiguousarray(np.asarray(beta, np.float32))
        in_maps.append(m)
    return in_maps


def kernel(x, adj_bool, node_mask, W, a_l, a_r, gamma, beta):
    global LAST_EXEC_TIME_NS, LAST_MEAN_EXEC_TIME_NS
    from concourse.bass_utils import run_bass_kernel_spmd

    gamma_np = np.asarray(gamma, dtype=np.float32)
    beta_np = np.asarray(beta, dtype=np.float32)
    apply_affine = not (np.all(gamma_np == 1.0) and np.all(beta_np == 0.0))

    nc = _get_program(apply_affine)
    in_maps = _prep_inputs(x, adj_bool, node_mask, W, a_l, a_r,
                           gamma_np, beta_np, apply_affine)
    trace = bool(int(os.environ.get("GAT_TRACE", "0")))
    res = run_bass_kernel_spmd(nc, in_maps, list(range(NCORES)), trace=trace)
    LAST_EXEC_TIME_NS = res.exec_time_ns
    LAST_MEAN_EXEC_TIME_NS = res.mean_exec_time_ns
    out = np.stack([np.asarray(r["out"], dtype=np.float32) for r in res.results])
    return out
